# revision 1
# baseline (speedup 1.0000x reference)
"""TRN2 Bass kernel for nn_DecoderLayer_70781061038465 (Falcon-7B style decoder
layer: fractured LayerNorm -> parallel MQA attention + MLP -> residual).

Sharding: 8-way tensor parallelism, no collectives. Each core computes a
partial sum of (attn_out + mlp_out) over its head/MLP shard; the host reduces
the 8 partials and adds the residual.

Per-core math (all LN work folded into matmuls):
  - LN affine folded into projection weights (columns scaled by ln_w; ln_b
    enters via a bias row consumed by an all-ones contraction row).
  - mean/rstd correction folded via (a) pre-scaling token rows by rstd and
    (b) a -mu*rstd contraction row whose weight-row is the column-sum of the
    ln_w-scaled weights.
  - softmax 1/sqrt(64) folded into wq.

Attention runs fully transposed: scoresT[sk,sq] come straight off the PE,
exp is applied without max-subtraction (score range is bounded for this
problem), the softmax denominator rides along as an all-ones 65th column of
V, and normalization happens on the token-major context eviction. No
probability transposes at all. The only XBAR transposes are: x-tilde
(wide DRAM->SBUF), per-head-pair context, and V.
"""
import sys
if "/opt/trn_rl_repo" not in sys.path:
    sys.path.insert(0, "/opt/trn_rl_repo")

from contextlib import ExitStack

import numpy as np
import ml_dtypes

import concourse.bass as bass
import concourse.tile as tile
from concourse import bacc, mybir
from concourse.bass_utils import run_bass_kernel_spmd

F32 = mybir.dt.float32
BF16 = mybir.dt.bfloat16
AX = mybir.AxisListType.X
AF = mybir.ActivationFunctionType
MUL = mybir.AluOpType.mult

# problem shapes (hardcoded per contract)
B, S, H, NH, HD = 2, 1024, 4544, 71, 64
T = B * S                 # 2048 tokens
HP = 4608                 # padded hidden (36*128)
KT = HP // 128            # 36 contraction tiles
NHP = 80                  # padded heads total
NHC = 10                  # heads per core
QC = NHC * HD             # 640 q channels/core
F4 = 4 * H                # 18176
F4C_REAL = F4 // 8        # 2272
F4C = 2304                # padded (18*128)
OC = QC + 128 + F4C       # 3072 proj out channels (q | k,v | h4h)
MT = OC // 128            # 24 proj m-tiles
DDK = (QC + F4C) // 128   # 23 dense+down contraction tiles
FC = HP // 512            # 9 output f-chunks
EPS = 1e-5
NEG = -30.0

_CACHE = {}


def _build():
    nc = bacc.Bacc("TRN2", target_bir_lowering=False, debug=False)
    xb_d = nc.dram_tensor("xb", [T, HP], BF16, kind="ExternalInput")
    wpk_d = nc.dram_tensor("wpk", [HP, OC], BF16, kind="ExternalInput")
    wdd_d = nc.dram_tensor("wdd", [QC + F4C, HP], BF16, kind="ExternalInput")
    cs_d = nc.dram_tensor("csn", [2, 128, S], BF16, kind="ExternalInput")
    dm_d = nc.dram_tensor("dmask", [128, 128], F32, kind="ExternalInput")
    out_d = nc.dram_tensor("out", [T, HP], F32, kind="ExternalOutput")

    xb = xb_d.ap()
    wpk = wpk_d.ap().rearrange("(ko p) c -> p ko c", p=128)   # [128, 36, 3072]
    wdd = wdd_d.ap().rearrange("(ko p) f -> p ko f", p=128)   # [128, 23, 4608]
    out = out_d.ap()

    with tile.TileContext(nc) as tc, ExitStack() as ctx:
        def pool(name, bufs, space="SBUF"):
            return ctx.enter_context(tc.tile_pool(name=name, bufs=bufs, space=space))

        const = pool("const", 1)
        xin = pool("xin", 1)
        xtp = pool("xtp", 1)
        wpool = pool("wp", 2)
        res = pool("res", 1)      # per-batch residents: qt/kt/vt/gt/ct
        et_p = pool("et", 1)
        c2_p = pool("c2", 2)
        wdp = pool("wdp", 3)
        outp = pool("outp", 1)
        tmp2 = pool("tmp2", 1)    # rot / vtmp
        small = pool("small", 4)
        psp = pool("psp", 8, space="PSUM")

        cos_sb = const.tile([128, S], BF16, tag="cos")
        nc.sync.dma_start(cos_sb[:], cs_d.ap()[0])
        sin_sb = const.tile([128, S], BF16, tag="sin")
        nc.sync.dma_start(sin_sb[:], cs_d.ap()[1])
        dmaskT = const.tile([128, 128], F32, tag="dmaskT")
        nc.sync.dma_start(dmaskT[:], dm_d.ap())

        for b in range(B):
            qt = res.tile([64, NHC, S], BF16, tag="qt")
            kt = res.tile([64, S], BF16, tag="kt")
            vt = res.tile([128, 8, 72], BF16, tag="vt")
            gt = res.tile([128, 18, S], BF16, tag="gt")
            ct = res.tile([128, 5, S], BF16, tag="ct")
            nc.vector.memset(vt[:, :, 64:65], 1.0)   # denominator ones-column

            # ---- Phase A: LN stats + rstd-scale, spill, wide transpose ----
            xt = xtp.tile([128, KT, S], BF16, tag="xt")
            for r in range(8):
                row0 = b * S + r * 128
                xrow = xin.tile([128, HP], BF16, tag="xrow")
                nc.sync.dma_start(xrow[:], xb[row0:row0 + 128, :])
                st = small.tile([128, 16, 6], F32, tag="st")
                xg = xrow[:, :H].rearrange("p (g d) -> p g d", g=16)
                for g in range(16):
                    nc.vector.bn_stats(st[:, g, :], xg[:, g, :])
                mv = small.tile([128, 2], F32, tag="mv")
                nc.vector.bn_aggr(mv[:], st[:])
                rstd = small.tile([128, 1], F32, tag="rstd")
                nc.vector.tensor_scalar_add(rstd[:], mv[:, 1:2], EPS)
                nc.scalar.activation(rstd[:], rstd[:], AF.Sqrt)
                nc.vector.reciprocal(rstd[:], rstd[:])
                mr = small.tile([128, 1], F32, tag="mr")
                nc.vector.tensor_tensor(mr[:], mv[:, 0:1], rstd[:], op=MUL)
                nc.vector.tensor_scalar_mul(mr[:], mr[:], -1.0)
                nc.vector.tensor_scalar_mul(xrow[:, :H], xrow[:, :H], rstd[:])
                nc.vector.memset(xrow[:, H:H + 1], 1.0)
                nc.vector.tensor_copy(xrow[:, H + 1:H + 2], mr[:])
                for k in range(KT):
                    nc.scalar.dma_start(
                        xt[:, k, r * 128:(r + 1) * 128],
                        xrow[:, k * 128:(k + 1) * 128], transpose=True)

            # ---- Phase B: projections (feature-major q/k/g, token-major v) ----
            for m in range(MT):
                wt = wpool.tile([128, KT, 128], BF16, tag="wt")
                nc.sync.dma_start(wt[:], wpk[:, :, m * 128:(m + 1) * 128])
                for hb in range(2):
                    hcols = slice(hb * 512, hb * 512 + 512)
                    ps = psp.tile([128, 512], F32, tag="ps",
                                  name=f"ps_{b}_{m}_{hb}")
                    for k in range(KT):
                        nc.tensor.matmul(ps[:], wt[:, k, :], xt[:, k, hcols],
                                         start=(k == 0), stop=(k == KT - 1))
                    if m < 5:
                        nc.vector.tensor_copy(qt[:, 2 * m, hcols], ps[:64, :])
                        nc.vector.tensor_copy(qt[:, 2 * m + 1, hcols],
                                              ps[64:128, :])
                    elif m == 5:
                        nc.vector.tensor_copy(kt[:, hcols], ps[:64, :])
                        for j in range(4):
                            r2 = hb * 4 + j
                            pv = psp.tile([128, 72], F32, tag="ps",
                                          name=f"pv_{b}_{r2}")
                            for k in range(KT):
                                nc.tensor.matmul(
                                    pv[:, :64],
                                    xt[:, k, r2 * 128:(r2 + 1) * 128],
                                    wt[:, k, 64:128],
                                    start=(k == 0), stop=(k == KT - 1))
                            nc.vector.tensor_copy(vt[:, r2, :64], pv[:, :64])
                    else:
                        nc.scalar.activation(gt[:, m - 6, hcols], ps[:], AF.Gelu)

            # ---- ROPE on qT (10 head tiles) and kT ----
            for mq in range(NHC + 1):
                tgt = qt[:, mq, :] if mq < NHC else kt[:]
                rot = tmp2.tile([64, S], BF16, tag="rot")
                nc.vector.tensor_scalar_mul(rot[0:32, :], tgt[32:64, :], -1.0)
                nc.vector.tensor_copy(rot[32:64, :], tgt[0:32, :])
                nc.vector.tensor_mul(tgt, tgt, cos_sb[:64, :])
                nc.vector.tensor_mul(rot[:], rot[:], sin_sb[:64, :])
                nc.vector.tensor_add(tgt, tgt, rot[:])

            # ---- Phase C: attention, fully transposed ----
            for h in range(NHC):
                et = et_p.tile([128, 8, S], BF16, tag="et", name=f"et_{b}_{h}")
                for skt in range(8):
                    for sqc in range(skt // 4, 2):
                        sp = psp.tile([128, 512], F32, tag="ps",
                                      name=f"sp_{b}_{h}_{skt}_{sqc}")
                        nc.tensor.matmul(
                            sp[:], kt[:, skt * 128:(skt + 1) * 128],
                            qt[:, h, sqc * 512:(sqc + 1) * 512],
                            start=True, stop=True)
                        if skt // 4 == sqc:
                            lc = skt * 128 - sqc * 512
                            nc.vector.tensor_tensor(
                                sp[:, lc:lc + 128], sp[:, lc:lc + 128],
                                dmaskT[:], op=mybir.AluOpType.add)
                        nc.scalar.activation(
                            et[:, skt, sqc * 512:(sqc + 1) * 512], sp[:],
                            AF.Exp)
                if h % 2 == 0:
                    c2 = c2_p.tile([128, 8, 128], BF16, tag="c2",
                                   name=f"c2_{b}_{h}")
                for sqt in range(8):
                    cp = psp.tile([128, 72], F32, tag="ps",
                                  name=f"cp_{b}_{h}_{sqt}")
                    for skt in range(sqt + 1):
                        nc.tensor.matmul(
                            cp[:, :65],
                            et[:, skt, sqt * 128:(sqt + 1) * 128],
                            vt[:, skt, :65],
                            start=(skt == 0), stop=(skt == sqt))
                    recd = small.tile([128, 1], F32, tag="recd")
                    nc.vector.reciprocal(recd[:], cp[:, 64:65])
                    nc.vector.tensor_scalar_mul(
                        c2[:, sqt, (h % 2) * 64:(h % 2) * 64 + 64],
                        cp[:, :64], recd[:])
                if h % 2 == 1:
                    for sqt in range(8):
                        nc.scalar.dma_start(
                            ct[:, h // 2, sqt * 128:(sqt + 1) * 128],
                            c2[:, sqt, :], transpose=True)

            # ---- Phase D: dense + down, fused PSUM accumulation ----
            for fc in range(FC):
                fcols = slice(fc * 512, (fc + 1) * 512)
                pss = [psp.tile([128, 512], F32, tag="ps",
                                name=f"pd_{b}_{fc}_{i}") for i in range(8)]
                for kk in range(DDK):
                    wdt = wdp.tile([128, 512], BF16, tag="wdt")
                    nc.sync.dma_start(wdt[:], wdd[:, kk, fcols])
                    for r in range(8):
                        tcols = slice(r * 128, (r + 1) * 128)
                        lh = (ct[:, kk, tcols] if kk < 5
                              else gt[:, kk - 5, tcols])
                        nc.tensor.matmul(pss[r][:], lh, wdt[:],
                                         start=(kk == 0), stop=(kk == DDK - 1))
                for r in range(8):
                    osb = outp.tile([128, 512], F32, tag="osb")
                    nc.vector.tensor_copy(osb[:], pss[r][:])
                    nc.sync.dma_start(
                        out[b * S + r * 128: b * S + (r + 1) * 128, fcols],
                        osb[:])
    nc.compile()
    return nc


def _prep_inputs(hidden_states, cos, sin, ln_w1, ln_b1, ln_w2, ln_b2,
                 wq, wk, wv, w_dense, w_h4h, w_4hh):
    f32 = np.float32
    bf = ml_dtypes.bfloat16
    lnw = np.concatenate([np.asarray(ln_w1), np.asarray(ln_w2)]).astype(np.float64)
    lnb = np.concatenate([np.asarray(ln_b1), np.asarray(ln_b2)]).astype(np.float64)

    def pack(Wc, scale=1.0):
        # Wc [O, H] -> [HP, O] f32: ln-folded + bias row + colsum row + zero pad
        W64 = Wc.astype(np.float64) * scale
        Wp = W64 * lnw                      # [O, H]
        bias = W64 @ lnb                    # [O]
        cw = Wp.sum(axis=1)                 # [O]
        O = Wc.shape[0]
        outw = np.zeros((HP, O), f32)
        outw[:H] = Wp.T.astype(f32)
        outw[H] = bias.astype(f32)
        outw[H + 1] = cw.astype(f32)
        return outw

    X = np.asarray(hidden_states, f32).reshape(T, H)
    xb = np.zeros((T, HP), bf)
    xb[:, :H] = X.astype(bf)

    cos2 = np.asarray(cos, f32)[0, 0]       # [S, 64]
    sin2 = np.asarray(sin, f32)[0, 0]
    csn = np.zeros((2, 128, S), bf)
    csn[0] = np.tile(cos2.T, (2, 1)).astype(bf)
    csn[1] = np.tile(sin2.T, (2, 1)).astype(bf)

    # transposed causal mask for scoresT[sk, sq]: keep sk <= sq
    dmask = np.where(np.arange(128)[:, None] <= np.arange(128)[None, :],
                     0.0, NEG).astype(f32)

    wq_pad = np.zeros((NHP * HD, H), f32)
    wq_pad[:NH * HD] = np.asarray(wq, f32)
    wdT_pad = np.zeros((NHP * HD, H), f32)
    wdT_pad[:NH * HD] = np.asarray(w_dense, f32).T
    w14 = np.asarray(w_h4h, f32)
    w41T = np.asarray(w_4hh, f32).T         # [F4, H]

    in_maps = []
    for c in range(8):
        hs = slice(c * QC, (c + 1) * QC)
        fs = slice(c * F4C_REAL, (c + 1) * F4C_REAL)
        wpk = np.zeros((HP, OC), f32)
        wpk[:, :QC] = pack(wq_pad[hs], scale=0.125)
        wpk[:, QC:QC + 64] = pack(np.asarray(wk, f32))
        wpk[:, QC + 64:QC + 128] = pack(np.asarray(wv, f32))
        wpk[:, QC + 128:QC + 128 + F4C_REAL] = pack(w14[fs])
        wdd = np.zeros((QC + F4C, HP), f32)
        wdd[:QC, :H] = wdT_pad[hs]
        wdd[QC:QC + F4C_REAL, :H] = w41T[fs]
        in_maps.append({
            "xb": xb, "wpk": wpk.astype(bf), "wdd": wdd.astype(bf),
            "csn": csn, "dmask": dmask,
        })
    return in_maps


def kernel(hidden_states, attention_mask, cos, sin,
           ln_w1, ln_b1, ln_w2, ln_b2,
           wq, wk, wv, w_dense, w_h4h, w_4hh):
    if "nc" not in _CACHE:
        _CACHE["nc"] = _build()
    nc = _CACHE["nc"]
    in_maps = _prep_inputs(hidden_states, cos, sin, ln_w1, ln_b1, ln_w2, ln_b2,
                           wq, wk, wv, w_dense, w_h4h, w_4hh)
    res = run_bass_kernel_spmd(nc, in_maps, core_ids=list(range(8)))
    acc = np.zeros((T, H), np.float64)
    for r in res.results:
        acc += r["out"][:, :H].astype(np.float64)
    outv = (acc.astype(np.float32)
            + np.asarray(hidden_states, np.float32).reshape(T, H))
    return outv.reshape(B, S, H).astype(np.float32)



# revision 13
# speedup vs baseline: 1.5946x; 1.5946x over previous
"""TRN2 Bass kernel for nn_DecoderLayer_70781061038465 (Falcon-7B style decoder
layer: fractured LayerNorm -> parallel MQA attention + MLP -> residual).

Sharding: 8-way tensor parallelism, 9 head-slots per core (71 real + 1 pad),
MLP 4h split 2272/core. Each core emits a full-width partial of
16*(attn_out + mlp_out); the host sums partials, divides by 16, and adds the
residual.

Numerics plan (fits rel-err < 2e-2 with margin, measured 1.36e-2 in sim):
  - proj (q/k/v/4h) and MLP-down run in fp8e4 DoubleRow "tier-C": both weight
    and activation split hi+lo, computing Wh@xh + Wl@xh + Wh@xl (3 paired
    products per 2 k-tiles = 0.75 cycles/row vs bf16's 1.0).
  - dense (ct@wd) runs "tier-B": W hi+lo, ct single fp8 (0.5 cycles/row).
  - attention internals bf16. All fp8 tensors pre-scaled x16 into e4m3's
    normal range; the x16 is unwound via cos/sin tables (/16), activation
    scale= arguments, and a final /16 on the host reduction.

Layout plan (no on-device transposes of x): x is uploaded twice -- token-major
for LayerNorm stats (bn_stats) and hidden-major for the projection GEMM. The
per-token rstd is broadcast to a [128, T] row tile via tiny PE outer-products
and applied on the hidden-major tiles directly (LN mean/bias fold into extra
contraction rows as in the classic trick). Attention runs fully transposed
(scoresT straight off the PE, denominator as a 65th ones-column of V);
context returns to feature-major via PE-transposes. Phase-D partials go
PSUM -> DRAM by direct DMA.
"""
import sys
if "/opt/trn_rl_repo" not in sys.path:
    sys.path.insert(0, "/opt/trn_rl_repo")

from contextlib import ExitStack

import numpy as np
import ml_dtypes

import concourse.bass as bass
import concourse.tile as tile
from concourse import bacc, mybir
from concourse.bass_utils import run_bass_kernel_spmd

F32 = mybir.dt.float32
BF16 = mybir.dt.bfloat16
FP8 = mybir.dt.float8e4
AF = mybir.ActivationFunctionType
MUL = mybir.AluOpType.mult
SUB = mybir.AluOpType.subtract
ADD = mybir.AluOpType.add
DR = mybir.MatmulPerfMode.DoubleRow

# problem shapes (hardcoded per contract)
B, S, H, NH, HD = 2, 1024, 4544, 71, 64
T = B * S
HP = 4608                 # padded hidden (36*128)
KT = HP // 128            # 36 contraction k-tiles
NSLOT = 9                 # head slots per core (8*9 = 72 >= 71)
F4 = 4 * H
F4C = F4 // 8             # 2272 4h-features per core
GKT = 18                  # 4h k-tiles per core (18*128 = 2304)
CKT = 6                   # ct k-tiles (5 used: 10 slots * 64; k5 zero pad)
MT = 24                   # proj m-tiles: m0-4 q, m5 k|v, m6-23 4h
NHF = 18                  # phase-D output half-chunks of 256 (18*256 = 4608)
EPS = 1e-5
NEGM = -240.0             # causal mask pre-exp-scale (exp scale 1/8 -> -30)

_CACHE = {}


def _build():
    nc = bacc.Bacc("TRN2", target_bir_lowering=False, debug=False)
    xtd_d = nc.dram_tensor("xtd", [128, KT, T], BF16, kind="ExternalInput")
    xbd_d = nc.dram_tensor("xbd", [T, H], BF16, kind="ExternalInput")
    wpk_d = nc.dram_tensor("wpk", [128, MT, KT, 2, 128], FP8,
                           kind="ExternalInput")
    wdd_d = nc.dram_tensor("wdd", [128, NHF, 2, 24, 256], FP8,
                           kind="ExternalInput")
    csn_d = nc.dram_tensor("csn", [2, 128, S], BF16, kind="ExternalInput")
    dmk_d = nc.dram_tensor("dmk", [128, 128], F32, kind="ExternalInput")
    idn_d = nc.dram_tensor("idn", [128, 128], BF16, kind="ExternalInput")
    out_d = nc.dram_tensor("out", [T, HP], BF16, kind="ExternalOutput")

    xtd = xtd_d.ap()
    xbd = xbd_d.ap()
    wpk = wpk_d.ap()
    wdd = wdd_d.ap()
    out = out_d.ap()

    with tile.TileContext(nc) as tc, ExitStack() as ctx:
        def pool(name, bufs, space="SBUF"):
            return ctx.enter_context(tc.tile_pool(name=name, bufs=bufs, space=space))

        const = pool("const", 1)
        xinp = pool("xinp", 2)        # token-major halves for stats
        smp = pool("smp", 2)          # rstd/mr gather + transpose
        rbcp = pool("rbc", 1)
        xrawp = pool("xraw", 2)       # hidden-major raw ring
        t2k = pool("t2k", 2)          # bf16 [128,1024] temps (xtmp / rot)
        xhlp = pool("xhl", 1)
        wpp = pool("wpp", 3)          # proj weight half-k tiles
        qtp = pool("qt", 1)
        ktp = pool("kt", 1)
        vtp = pool("vt", 1)
        vtm = pool("vtm", 1)
        gtm = pool("gtm", 1)
        cgcp = pool("cgc", 1)
        cggp = pool("cgg", 1)
        etbp = pool("etb", 2)
        c2p = pool("c2", 1)
        wdtp = pool("wdt", 2)
        obp = pool("ob", 2)
        small = pool("small", 3)
        psp = pool("psp", 8, space="PSUM")

        cos_sb = const.tile([128, S], BF16, tag="cos")
        nc.sync.dma_start(cos_sb[:], csn_d.ap()[0])
        sin_sb = const.tile([128, S], BF16, tag="sin")
        nc.sync.dma_start(sin_sb[:], csn_d.ap()[1])
        dmaskT = const.tile([128, 128], F32, tag="dmaskT")
        nc.sync.dma_start(dmaskT[:], dmk_d.ap())
        idn = const.tile([128, 128], BF16, tag="idn")
        nc.sync.dma_start(idn[:], idn_d.ap())

        # persistent pads in xhl (written once; per-batch writes never touch)
        xhl = xhlp.tile([128, KT, 2, S], FP8, tag="xhl")
        nc.vector.memset(xhl[64:128, KT - 1, :, :], 0.0)
        nc.vector.memset(xhl[64:65, KT - 1, 0, :], 1.0)  # ones row (h=4544)

        cgc = cgcp.tile([128, CKT, S], FP8, tag="cgc")
        nc.vector.memset(cgc[:, 5, :], 0.0)              # dense pad k-tile
        nc.vector.memset(cgc[64:128, 4, :], 0.0)         # pad head slot 9

        for b in range(B):
            # ================= Phase A: LN stats -> rstd broadcast =========
            sm = smp.tile([128, 128], BF16, tag="sm", name=f"sm_{b}")
            for rt in range(8):
                row0 = b * S + rt * 128
                st = small.tile([128, 10, 6], F32, tag="st")
                for hh in range(2):
                    xin = xinp.tile([128, H // 2], BF16, tag="xin")
                    nc.sync.dma_start(
                        xin[:], xbd[row0:row0 + 128,
                                    hh * (H // 2):(hh + 1) * (H // 2)])
                    for g in range(4):
                        nc.vector.bn_stats(st[:, hh * 5 + g, :],
                                           xin[:, g * 512:(g + 1) * 512])
                    nc.vector.bn_stats(st[:, hh * 5 + 4, :], xin[:, 2048:2272])
                mv = small.tile([128, 2], F32, tag="mv")
                nc.vector.bn_aggr(mv[:], st[:])
                t1 = small.tile([128, 1], F32, tag="t1")
                nc.vector.tensor_scalar_add(t1[:], mv[:, 1:2], EPS)
                nc.scalar.activation(t1[:], t1[:], AF.Sqrt)
                t3 = small.tile([128, 1], F32, tag="t3")
                nc.vector.reciprocal(t3[:], t1[:])
                nc.vector.tensor_copy(sm[:, rt:rt + 1], t3[:])
                t2 = small.tile([128, 1], F32, tag="t2")
                nc.vector.tensor_tensor(t2[:], mv[:, 0:1], t3[:], op=MUL)
                nc.vector.tensor_scalar_mul(t2[:], t2[:], -16.0)
                nc.vector.tensor_copy(sm[:, 8 + rt:9 + rt], t2[:])
            smT = smp.tile([128, 128], BF16, tag="smT", name=f"smT_{b}")
            nc.sync.dma_start_transpose(smT[:], sm[:])
            rbc = rbcp.tile([128, S], BF16, tag="rbc", name=f"rbc_{b}")
            for rt in range(8):
                nc.gpsimd.partition_broadcast(
                    rbc[:, rt * 128:(rt + 1) * 128], smT[rt:rt + 1, :])
                # mr contraction row (h=4545): hi then lo residual
                nc.vector.tensor_copy(
                    xhl[65:66, KT - 1, 0, rt * 128:(rt + 1) * 128],
                    smT[8 + rt:9 + rt, :])
                nc.vector.tensor_tensor(
                    xhl[65:66, KT - 1, 1, rt * 128:(rt + 1) * 128],
                    smT[8 + rt:9 + rt, :],
                    xhl[65:66, KT - 1, 0, rt * 128:(rt + 1) * 128], op=SUB)

            # ============ Phase A2: scale hidden-major x, split hi/lo ======
            for k in range(KT):
                nreal = 64 if k == KT - 1 else 128
                xr = xrawp.tile([128, S], BF16, tag="xr")
                nc.sync.dma_start(xr[:], xtd[:, k, b * S:(b + 1) * S])
                xt_ = t2k.tile([128, S], BF16, tag="xtmp")
                nc.vector.tensor_tensor(xt_[:nreal, :], xr[:nreal, :],
                                        rbc[:nreal, :], op=MUL)
                nc.scalar.activation(xhl[:nreal, k, 0, :], xt_[:nreal, :],
                                     AF.Copy)
                nc.vector.tensor_tensor(xhl[:nreal, k, 1, :], xt_[:nreal, :],
                                        xhl[:nreal, k, 0, :], op=SUB)

            # ================= Phase B: fused projection ===================
            qt = qtp.tile([128, 5, S], BF16, tag="qt", name=f"qt_{b}")
            kt2 = ktp.tile([128, S], BF16, tag="kt2", name=f"kt2_{b}")
            vt = vtp.tile([128, 8, 65], BF16, tag="vt", name=f"vt_{b}")
            nc.vector.memset(vt[:, :, 64:65], 1.0)
            cgg = cggp.tile([128, GKT, 2, S], FP8, tag="cgg", name=f"cgg_{b}")
            for m in range(MT):
                wha = wpp.tile([128, KT // 2, 2, 128], FP8, tag="wt",
                               name=f"wha_{b}_{m}")
                nc.sync.dma_start(wha[:], wpk[:, m, 0:KT // 2, :, :])
                whb = wpp.tile([128, KT // 2, 2, 128], FP8, tag="wt",
                               name=f"whb_{b}_{m}")
                nc.sync.dma_start(whb[:], wpk[:, m, KT // 2:KT, :, :])
                for hc in range(2):
                    tcols = slice(hc * 512, hc * 512 + 512)
                    ps = psp.tile([128, 512], F32, tag="ps",
                                  name=f"pb_{b}_{m}_{hc}")
                    for kp in range(KT // 2):
                        k2 = slice(2 * kp, 2 * kp + 2)
                        wt = wha if kp < KT // 4 else whb
                        w2 = slice(2 * kp - (0 if kp < KT // 4 else KT // 2),
                                   2 * kp + 2 - (0 if kp < KT // 4 else KT // 2))
                        st_ = (kp == 0)
                        nc.tensor.matmul(ps[:], wt[:, w2, 0, :],
                                         xhl[:, k2, 0, tcols],
                                         start=st_, stop=False, perf_mode=DR)
                        nc.tensor.matmul(ps[:], wt[:, w2, 1, :],
                                         xhl[:, k2, 0, tcols],
                                         start=False, stop=False, perf_mode=DR)
                        nc.tensor.matmul(ps[:], wt[:, w2, 0, :],
                                         xhl[:, k2, 1, tcols],
                                         start=False, stop=(kp == KT // 2 - 1),
                                         perf_mode=DR)
                    if m < 5:
                        nc.vector.tensor_copy(qt[:, m, tcols], ps[:])
                    elif m == 5:
                        nc.vector.tensor_copy(kt2[0:64, tcols], ps[0:64, :])
                        vtmp = vtm.tile([64, 512], BF16, tag="vtmp")
                        nc.vector.tensor_copy(vtmp[:], ps[64:128, :])
                        for j in range(4):
                            pv = psp.tile([128, 64], BF16, tag="ps",
                                          name=f"pv_{b}_{hc}_{j}")
                            nc.tensor.transpose(
                                pv[:], vtmp[:, j * 128:(j + 1) * 128],
                                idn[0:64, 0:64])
                            nc.scalar.activation(vt[:, hc * 4 + j, 0:64],
                                                 pv[:], AF.Copy,
                                                 scale=1.0 / 16.0)
                    else:
                        gt_ = gtm.tile([128, 512], BF16, tag="gt")
                        nc.scalar.activation(gt_[:], ps[:], AF.Gelu,
                                             scale=1.0 / 16.0)
                        nc.scalar.activation(cgg[:, m - 6, 0, tcols], gt_[:],
                                             AF.Copy)
                        nc.vector.tensor_tensor(cgg[:, m - 6, 1, tcols],
                                                gt_[:], cgg[:, m - 6, 0, tcols],
                                                op=SUB)
                if m < 5:
                    # rope both head slots of this m-tile, in place
                    rot = t2k.tile([128, S], BF16, tag="rot")
                    tgt = qt[:, m, :]
                    nc.vector.tensor_scalar_mul(rot[0:32, :], tgt[32:64, :], -1.0)
                    nc.vector.tensor_copy(rot[32:64, :], tgt[0:32, :])
                    nc.vector.tensor_scalar_mul(rot[64:96, :], tgt[96:128, :], -1.0)
                    nc.vector.tensor_copy(rot[96:128, :], tgt[64:96, :])
                    nc.vector.tensor_mul(tgt, tgt, cos_sb[:])
                    nc.vector.tensor_mul(rot[:], rot[:], sin_sb[:])
                    nc.vector.tensor_add(tgt, tgt, rot[:])
                elif m == 5:
                    rot = t2k.tile([128, S], BF16, tag="rot")
                    tgt = kt2[0:64, :]
                    nc.vector.tensor_scalar_mul(rot[0:32, :], kt2[32:64, :], -1.0)
                    nc.vector.tensor_copy(rot[32:64, :], kt2[0:32, :])
                    nc.vector.tensor_mul(tgt, tgt, cos_sb[0:64, :])
                    nc.vector.tensor_mul(rot[0:64, :], rot[0:64, :],
                                         sin_sb[0:64, :])
                    nc.vector.tensor_add(tgt, tgt, rot[0:64, :])
                    nc.vector.tensor_copy(kt2[64:128, :], kt2[0:64, :])

            # ================= Phase C: attention ==========================
            for h in range(NSLOT):
                qb_ = (h % 2) * 64
                kb = qb_
                c2 = None
                if h % 2 == 0:
                    c2 = c2p.tile([128, 8, 128], FP8, tag="c2",
                                  name=f"c2_{b}_{h}")
                    if h == 8:
                        nc.vector.memset(c2[:, :, 64:128], 0.0)
                else:
                    c2 = c2p.tile([128, 8, 128], FP8, tag="c2",
                                  name=f"c2_{b}_{h - 1}")
                for sqc in range(2):
                    nsk = 4 if sqc == 0 else 8
                    et = etbp.tile([128, nsk, 512], BF16, tag=f"et{sqc}",
                                   name=f"et_{b}_{h}_{sqc}",
                                   bufs=1 if sqc == 0 else 2)
                    scols = slice(sqc * 512, sqc * 512 + 512)
                    for skt in range(nsk):
                        sp = psp.tile([128, 512], F32, tag="ps",
                                      name=f"sp_{b}_{h}_{sqc}_{skt}")
                        nc.tensor.matmul(
                            sp[:], kt2[kb:kb + 64, skt * 128:(skt + 1) * 128],
                            qt[qb_:qb_ + 64, h // 2, scols],
                            start=True, stop=True)
                        dg = skt - sqc * 4
                        if dg >= 0:
                            nc.vector.tensor_tensor(
                                sp[:, dg * 128:(dg + 1) * 128],
                                sp[:, dg * 128:(dg + 1) * 128],
                                dmaskT[:], op=ADD)
                        nc.scalar.activation(et[:, skt, :], sp[:], AF.Exp,
                                             scale=0.125)
                    for sqt in range(4):
                        gq = sqc * 4 + sqt
                        cp = psp.tile([128, 72], F32, tag="ps",
                                      name=f"cp_{b}_{h}_{gq}")
                        for skt in range(gq + 1):
                            nc.tensor.matmul(cp[:, :65], et[:, skt,
                                                            sqt * 128:(sqt + 1) * 128],
                                             vt[:, skt, :65],
                                             start=(skt == 0), stop=(skt == gq))
                        recd = small.tile([128, 1], F32, tag="recd")
                        nc.vector.reciprocal(recd[:], cp[:, 64:65])
                        nc.scalar.activation(c2[:, gq, qb_:qb_ + 64],
                                             cp[:, :64], AF.Copy, scale=recd[:])
                if h % 2 == 1 or h == 8:
                    for gq in range(8):
                        pt = psp.tile([128, 128], FP8, tag="ps",
                                      name=f"pt_{b}_{h}_{gq}")
                        nc.tensor.transpose(pt[:], c2[:, gq, :], idn[:])
                        nc.vector.tensor_copy(
                            cgc[:, h // 2, gq * 128:(gq + 1) * 128], pt[:])

            # ================= Phase D: dense + down -> DRAM ===============
            for hf in range(NHF):
                wda = wdtp.tile([128, 24, 256], FP8, tag="wd",
                                name=f"wda_{b}_{hf}")
                nc.sync.dma_start(wda[:], wdd[:, hf, 0, :, :])
                wdb = wdtp.tile([128, 24, 256], FP8, tag="wd",
                                name=f"wdb_{b}_{hf}")
                nc.sync.dma_start(wdb[:], wdd[:, hf, 1, :, :])
                fcols = slice(hf * 256, hf * 256 + 256)
                for r in range(8):
                    tcols = slice(r * 128, r * 128 + 128)
                    pd = psp.tile([128, 256], F32, tag="ps",
                                  name=f"pd_{b}_{hf}_{r}")
                    for kp in range(3):
                        k2 = slice(2 * kp, 2 * kp + 2)
                        nc.tensor.matmul(pd[:], cgc[:, k2, tcols],
                                         wda[:, 2 * kp:2 * kp + 2, :],
                                         start=(kp == 0), stop=False,
                                         perf_mode=DR)
                        nc.tensor.matmul(pd[:], cgc[:, k2, tcols],
                                         wda[:, 6 + 2 * kp:8 + 2 * kp, :],
                                         start=False, stop=False, perf_mode=DR)
                    for kp in range(9):
                        k2 = slice(2 * kp, 2 * kp + 2)
                        if kp < 3:
                            whi = wda[:, 12 + 2 * kp:14 + 2 * kp, :]
                            wlo = wda[:, 18 + 2 * kp:20 + 2 * kp, :]
                        else:
                            whi = wdb[:, 2 * (kp - 3):2 * (kp - 3) + 2, :]
                            wlo = wdb[:, 12 + 2 * (kp - 3):14 + 2 * (kp - 3), :]
                        nc.tensor.matmul(pd[:], cgg[:, k2, 0, tcols], whi,
                                         start=False, stop=False, perf_mode=DR)
                        nc.tensor.matmul(pd[:], cgg[:, k2, 0, tcols], wlo,
                                         start=False, stop=False, perf_mode=DR)
                        nc.tensor.matmul(pd[:], cgg[:, k2, 1, tcols], whi,
                                         start=False, stop=(kp == 8),
                                         perf_mode=DR)
                    ob = obp.tile([128, 256], BF16, tag="ob")
                    if r % 2 == 0:
                        nc.vector.tensor_copy(ob[:], pd[:])
                    else:
                        nc.scalar.activation(ob[:], pd[:], AF.Copy)
                    nc.sync.dma_start(
                        out[b * S + r * 128:b * S + (r + 1) * 128, fcols],
                        ob[:])
    nc.compile()
    return nc


def _prep_inputs(hidden_states, cos, sin, ln_w1, ln_b1, ln_w2, ln_b2,
                 wq, wk, wv, w_dense, w_h4h, w_4hh):
    f32 = np.float32
    bf = ml_dtypes.bfloat16
    f8 = ml_dtypes.float8_e4m3fn
    lnw = np.concatenate([np.asarray(ln_w1), np.asarray(ln_w2)]).astype(np.float64)
    lnb = np.concatenate([np.asarray(ln_b1), np.asarray(ln_b2)]).astype(np.float64)

    X = np.asarray(hidden_states, f32).reshape(T, H).astype(bf)
    xbd = np.ascontiguousarray(X)                        # [T, H] bf16
    xtf = np.zeros((HP, T), bf)
    xtf[:H] = X.T
    xtd = np.ascontiguousarray(
        xtf.reshape(KT, 128, T).transpose(1, 0, 2))      # [128, KT, T]

    def pack16(W):
        # W [O, H] -> ln-folded, bias + colsum/16 rows, x16: [O, HP] f32
        W64 = W.astype(np.float64) * 16.0
        out_ = np.zeros((W.shape[0], HP), np.float64)
        out_[:, :H] = W64 * lnw
        out_[:, H] = W64 @ lnb
        out_[:, H + 1] = out_[:, :H].sum(1) / 16.0
        return out_.astype(f32)

    def hilo(Wp):
        hi = Wp.astype(f8)
        lo = (Wp - hi.astype(f32)).astype(f8)
        return hi, lo

    wq_f = np.asarray(wq, f32)          # [NH*HD, H]
    wk_f = np.asarray(wk, f32)
    wv_f = np.asarray(wv, f32)
    w14 = np.asarray(w_h4h, f32)        # [F4, H]
    wdT = np.asarray(w_dense, f32).T    # [NH*HD, H]
    w41T = np.asarray(w_4hh, f32).T     # [F4, H]

    cos2 = np.asarray(cos, f32)[0, 0] / 16.0   # [S, 64]
    sin2 = np.asarray(sin, f32)[0, 0] / 16.0
    csn = np.zeros((2, 128, S), bf)
    csn[0] = np.tile(cos2.T, (2, 1)).astype(bf)
    csn[1] = np.tile(sin2.T, (2, 1)).astype(bf)
    dmk = np.where(np.arange(128)[:, None] <= np.arange(128)[None, :],
                   0.0, NEGM).astype(f32)
    idn = np.eye(128, dtype=bf)

    in_maps = []
    for c in range(8):
        # --- projection weights [O=3072 rows, HP] ---
        Wall = np.zeros((MT * 128, H), f32)
        for s in range(10):
            gh = c * NSLOT + s
            if s < NSLOT and gh < NH:
                Wall[s * 64:(s + 1) * 64] = wq_f[gh * HD:(gh + 1) * HD]
        Wall[5 * 128:5 * 128 + 64] = wk_f
        Wall[5 * 128 + 64:6 * 128] = wv_f
        f0 = c * F4C
        Wall[6 * 128:6 * 128 + F4C] = w14[f0:f0 + F4C]
        Wp = pack16(Wall)
        hi, lo = hilo(Wp)
        # [O, HP] -> [128(p), MT, KT, 2, 128(j)]
        def swz(a):
            return a.reshape(MT, 128, KT, 128).transpose(3, 0, 2, 1)
        wpk = np.ascontiguousarray(
            np.stack([swz(hi), swz(lo)], axis=3))        # [128,MT,KT,2,128]

        # --- phase-D weights ---
        # dense rows laid out in ct order: feature f = pair*128 + within,
        # slot = pair*2 + (within>=64), d = within%64
        Wd = np.zeros((CKT * 128, HP), f32)
        for s in range(NSLOT):
            gh = c * NSLOT + s
            if gh >= NH:
                continue
            pair, half = divmod(s, 2)
            Wd[pair * 128 + half * 64:pair * 128 + half * 64 + 64, :H] = \
                (wdT[gh * HD:(gh + 1) * HD] * 16.0)
        W4 = np.zeros((GKT * 128, HP), f32)
        W4[:F4C, :H] = w41T[f0:f0 + F4C] * 16.0
        dh, dl = hilo(Wd)
        gh_, gl_ = hilo(W4)

        wddc = np.zeros((128, NHF, 2, 24, 256), f8)
        for hf in range(NHF):
            cols = slice(hf * 256, hf * 256 + 256)
            def kt_rows(a, k):
                return a[k * 128:(k + 1) * 128, cols]    # [128, 256]
            for k in range(6):
                wddc[:, hf, 0, k] = kt_rows(dh, k)
                wddc[:, hf, 0, 6 + k] = kt_rows(dl, k)
                wddc[:, hf, 0, 12 + k] = kt_rows(gh_, k)
                wddc[:, hf, 0, 18 + k] = kt_rows(gl_, k)
            for k in range(12):
                wddc[:, hf, 1, k] = kt_rows(gh_, 6 + k)
                wddc[:, hf, 1, 12 + k] = kt_rows(gl_, 6 + k)
        in_maps.append({
            "xtd": xtd, "xbd": xbd, "wpk": wpk.astype(f8),
            "wdd": wddc, "csn": csn, "dmk": dmk, "idn": idn,
        })
    return in_maps


def kernel(hidden_states, attention_mask, cos, sin,
           ln_w1, ln_b1, ln_w2, ln_b2,
           wq, wk, wv, w_dense, w_h4h, w_4hh):
    if "nc" not in _CACHE:
        _CACHE["nc"] = _build()
    nc = _CACHE["nc"]
    in_maps = _prep_inputs(hidden_states, cos, sin, ln_w1, ln_b1, ln_w2, ln_b2,
                           wq, wk, wv, w_dense, w_h4h, w_4hh)
    res = run_bass_kernel_spmd(nc, in_maps, core_ids=list(range(8)))
    acc = np.zeros((T, H), np.float64)
    for r in res.results:
        acc += r["out"][:, :H].astype(np.float64)
    outv = (acc / 16.0).astype(np.float32) \
        + np.asarray(hidden_states, np.float32).reshape(T, H)
    return outv.reshape(B, S, H).astype(np.float32)
